# revision 1
# baseline (speedup 1.0000x reference)
"""HFreqC layer kernel for 8 Trainium2 NeuronCores.

The reference op (FFT -> zero centered low-freq band -> IFFT -> real -> relu)
is, up to the relu, a fixed real linear operator along the channel axis:
    y = relu(x @ W),  W = Re(ifft(mask * fft(I)))^T   (728x728, symmetric)

Strategy: pure data parallel over rows (32*38*38 = 46208 rows, 5776/core,
padded to 6144 = 12 groups of 512 rows). The host shards rows across the 8
cores and lays each shard out channel-major (transposed) while padding, so
the device reads are all contiguous. Per core:
  - W (row-padded to 768) lives in SBUF as 6 k-tiles [128, 728].
  - Each 512-row group is one contiguous [128, 3072] DMA holding X^T tiles
    [128ch x 128row] for (g in 4 row-tiles) x (u in 6 k-tiles).
  - fp32 matmuls in float32r mode (1 cycle/row at N>=256), accumulating
    over 6 k-tiles into PSUM, j in two 364-wide chunks.
  - ScalarE applies relu on the PSUM->SBUF copy; contiguous DMA out.
"""

import numpy as np

C = 728            # channels
KT = 6             # k tiles of 128 (channel pad to 768)
CP = KT * 128      # 768 padded channels
G = 4              # row-tiles (128 rows) per group
GROUP_ROWS = 128 * G
N_CORES = 8
ROWS_TOTAL = 32 * 38 * 38          # 46208
ROWS_PER_CORE = ROWS_TOTAL // N_CORES  # 5776
N_GROUPS = 12
ROWS_PAD = N_GROUPS * GROUP_ROWS   # 6144
JC = 364           # j-chunk width (2 chunks of 364; both >=256 for f32r rate)

_CACHE = {}


def _build_w(scale: int) -> np.ndarray:
    """[CP, C] f32: W padded with zero rows; y_row = x_row @ W."""
    m_sh = np.ones(C)
    m_sh[C // 2 - C // scale: C // 2 + C // scale] = 0
    m = np.fft.ifftshift(m_sh)
    A = np.fft.ifft(m[:, None] * np.fft.fft(np.eye(C), axis=0), axis=0)
    W = np.real(A).T.astype(np.float32)
    Wp = np.zeros((CP, C), dtype=np.float32)
    Wp[:C] = W
    return Wp


def _shard_xt(xf: np.ndarray, core: int) -> np.ndarray:
    """[N_GROUPS, 128, G*CP]: [grp][p][g*CP + u*128 + m] = x[512grp+128g+m, 128u+p]."""
    xp = np.zeros((ROWS_PAD, CP), dtype=np.float32)
    xp[:ROWS_PER_CORE, :C] = xf[core * ROWS_PER_CORE:(core + 1) * ROWS_PER_CORE]
    v = xp.reshape(N_GROUPS, G, 128, KT, 128)          # grp g m u p
    v = v.transpose(0, 4, 1, 3, 2)                     # grp p g u m
    return np.ascontiguousarray(v).reshape(N_GROUPS, 128, G * CP)


def _build_nc(repeat: int = 1):
    import concourse.mybir as mybir
    import concourse.tile as tile
    from concourse import bacc

    fp32 = mybir.dt.float32
    fp32r = mybir.dt.float32r

    nc = bacc.Bacc("TRN2", target_bir_lowering=False)
    x_d = nc.dram_tensor("x", [N_GROUPS, 128, G * CP], fp32r, kind="ExternalInput").ap()
    w_d = nc.dram_tensor("w", [CP, C], fp32r, kind="ExternalInput").ap()
    y_d = nc.dram_tensor("y", [ROWS_PAD, C], fp32, kind="ExternalOutput").ap()

    y_v = y_d.rearrange("(grp g p) j -> grp p g j", g=G, p=128)
    w_v = w_d.rearrange("(u p) j -> p u j", u=KT, p=128)

    with tile.TileContext(nc) as tc:
        with (
            tc.tile_pool(name="wpool", bufs=1) as wpool,
            tc.tile_pool(name="io", bufs=4) as io,
            tc.tile_pool(name="psp", bufs=8, space="PSUM") as psp,
        ):
            w_sb = wpool.tile([128, KT * C], fp32r)
            nc.sync.dma_start(out=w_sb.rearrange("p (u j) -> p u j", u=KT, j=C), in_=w_v)
            for _it in range(N_GROUPS * repeat):
                grp = _it % N_GROUPS
                xt = io.tile([128, G * CP], fp32r, tag="xt")
                half = G * CP // 2
                nc.sync.dma_start(out=xt[:, :half], in_=x_d[grp, :, :half])
                nc.sync.dma_start(out=xt[:, half:], in_=x_d[grp, :, half:])
                ysb = io.tile([128, G * C], fp32, tag="y")
                for g in range(G):
                    for jc in range(2):
                        j0 = jc * JC
                        ps = psp.tile([128, JC], fp32, tag="ps")
                        for u in range(KT):
                            nc.tensor.matmul(
                                ps,
                                lhsT=xt[:, g * CP + u * 128: g * CP + (u + 1) * 128],
                                rhs=w_sb[:, u * C + j0: u * C + j0 + JC],
                                start=(u == 0),
                                stop=(u == KT - 1),
                            )
                        nc.scalar.activation(
                            ysb[:, g * C + j0: g * C + j0 + JC],
                            ps,
                            mybir.ActivationFunctionType.Relu,
                        )
                ysb_v = ysb.rearrange("p (g j) -> p g j", g=G, j=C)
                nc.scalar.dma_start(out=y_v[grp][:, 0:2], in_=ysb_v[:, 0:2])
                nc.scalar.dma_start(out=y_v[grp][:, 2:4], in_=ysb_v[:, 2:4])
    nc.compile()
    return nc


def _make_in_maps(x: np.ndarray, scale: int):
    xf = np.asarray(x, dtype=np.float32).reshape(-1, C)
    W = _build_w(scale)
    return [{"x": _shard_xt(xf, i), "w": W} for i in range(N_CORES)]


def kernel(x: np.ndarray, scale) -> np.ndarray:
    import sys
    if "/opt/trn_rl_repo" not in sys.path:
        sys.path.insert(0, "/opt/trn_rl_repo")
    from concourse.bass_utils import run_bass_kernel_spmd

    scale = int(np.asarray(scale))
    x = np.asarray(x, dtype=np.float32)
    orig_shape = x.shape

    if "nc" not in _CACHE:
        _CACHE["nc"] = _build_nc()
    nc = _CACHE["nc"]

    in_maps = _make_in_maps(x, scale)
    res = run_bass_kernel_spmd(nc, in_maps, list(range(N_CORES)))
    outs = [r["y"][:ROWS_PER_CORE] for r in res.results]
    y = np.concatenate(outs, axis=0).reshape(orig_shape)
    return y.astype(np.float32)



# revision 2
# speedup vs baseline: 13.2229x; 13.2229x over previous
"""HFreqC layer kernel for Trainium2 (axon-tunneled NeuronCores).

The reference op (FFT -> zero centered low-freq band -> IFFT -> real -> relu)
is, up to the relu, a fixed real linear operator along the channel axis:
    y = relu(x @ W),  W = Re(ifft(mask * fft(I)))^T   (728x728, circulant)

Strategy: pure data parallel over rows (32*38*38 = 46208 rows). The host
shards rows across N_CORES cores, lays each shard out channel-major
(transposed, zero-padded to 768 channels / 512-row groups) and casts to
bf16, so all device reads are contiguous and HBM traffic is halved; fp32
PSUM accumulation keeps the error at ~2e-3. Per core:
  - W (row-padded to 768, bf16) lives in SBUF as 6 k-tiles [128, 728].
  - Each 512-row group is one contiguous [128, 3072] DMA holding X^T tiles
    [128ch x 128row] for (g in 4 row-tiles) x (u in 6 k-tiles).
  - bf16 matmuls accumulate over 6 k-tiles into fp32 PSUM, j in two
    364-wide chunks; ScalarE applies relu on the PSUM->SBUF copy (bf16).
  - Contiguous bf16 DMA out; host casts back to fp32.

N_CORES is chosen for end-to-end throughput: per-execute dispatch cost
through the axon tunnel scales with the number of per-core executes, while
on-core time (~0.1 ms at 8 cores) is small against it, so fewer, larger
shards win.
"""

import numpy as np

C = 728                 # channels
KT = 6                  # k tiles of 128 (channel pad to 768)
CP = KT * 128           # 768 padded channels
G = 4                   # row-tiles (128 rows) per group
GROUP_ROWS = 128 * G    # 512
ROWS_TOTAL = 32 * 38 * 38   # 46208
JC = 364                # j-chunk width (2 chunks of 364)
N_CORES = 4

_CACHE = {}


def _bf16():
    import ml_dtypes
    return np.dtype(ml_dtypes.bfloat16)


def _n_groups(n_cores: int) -> int:
    return -(-(ROWS_TOTAL // n_cores) // GROUP_ROWS)


def _build_w(scale: int) -> np.ndarray:
    """[CP, C] bf16: W padded with zero rows; y_row = x_row @ W."""
    m_sh = np.ones(C)
    m_sh[C // 2 - C // scale: C // 2 + C // scale] = 0
    m = np.fft.ifftshift(m_sh)
    A = np.fft.ifft(m[:, None] * np.fft.fft(np.eye(C), axis=0), axis=0)
    W = np.real(A).T.astype(np.float32)
    Wp = np.zeros((CP, C), dtype=np.float32)
    Wp[:C] = W
    return Wp.astype(_bf16())


def _shard_xt(xf: np.ndarray, core: int, n_cores: int) -> np.ndarray:
    """[ng, 128, G*CP] bf16: [grp][p][g*CP+u*128+m] = x[512grp+128g+m, 128u+p]."""
    ng = _n_groups(n_cores)
    rows = ROWS_TOTAL // n_cores
    xp = np.zeros((ng * GROUP_ROWS, CP), dtype=np.float32)
    xp[:rows, :C] = xf[core * rows:(core + 1) * rows]
    v = xp.reshape(ng, G, 128, KT, 128)      # grp g m u p
    v = v.transpose(0, 4, 1, 3, 2)           # grp p g u m
    return np.ascontiguousarray(v).astype(_bf16()).reshape(ng, 128, G * CP)


def _build_nc(n_cores: int):
    import concourse.mybir as mybir
    import concourse.tile as tile
    from concourse import bacc

    ng = _n_groups(n_cores)
    bf16 = mybir.dt.bfloat16
    fp32 = mybir.dt.float32

    nc = bacc.Bacc("TRN2", target_bir_lowering=False)
    x_d = nc.dram_tensor("x", [ng, 128, G * CP], bf16, kind="ExternalInput").ap()
    w_d = nc.dram_tensor("w", [CP, C], bf16, kind="ExternalInput").ap()
    y_d = nc.dram_tensor("y", [ng * GROUP_ROWS, C], bf16, kind="ExternalOutput").ap()

    y_v = y_d.rearrange("(grp g p) j -> grp p g j", g=G, p=128)
    w_v = w_d.rearrange("(u p) j -> p u j", u=KT, p=128)

    with tile.TileContext(nc) as tc:
        with (
            tc.tile_pool(name="wpool", bufs=1) as wpool,
            tc.tile_pool(name="io", bufs=4) as io,
            tc.tile_pool(name="psp", bufs=8, space="PSUM") as psp,
        ):
            w_sb = wpool.tile([128, KT * C], bf16)
            nc.sync.dma_start(out=w_sb.rearrange("p (u j) -> p u j", u=KT, j=C), in_=w_v)
            for grp in range(ng):
                xt = io.tile([128, G * CP], bf16, tag="xt")
                half = G * CP // 2
                nc.sync.dma_start(out=xt[:, :half], in_=x_d[grp, :, :half])
                nc.sync.dma_start(out=xt[:, half:], in_=x_d[grp, :, half:])
                ysb = io.tile([128, G * C], bf16, tag="y")
                for g in range(G):
                    for jc in range(2):
                        j0 = jc * JC
                        ps = psp.tile([128, JC], fp32, tag="ps")
                        for u in range(KT):
                            nc.tensor.matmul(
                                ps,
                                lhsT=xt[:, g * CP + u * 128: g * CP + (u + 1) * 128],
                                rhs=w_sb[:, u * C + j0: u * C + j0 + JC],
                                start=(u == 0),
                                stop=(u == KT - 1),
                            )
                        nc.scalar.activation(
                            ysb[:, g * C + j0: g * C + j0 + JC],
                            ps,
                            mybir.ActivationFunctionType.Relu,
                        )
                ysb_v = ysb.rearrange("p (g j) -> p g j", g=G, j=C)
                nc.scalar.dma_start(out=y_v[grp][:, 0:2], in_=ysb_v[:, 0:2])
                nc.scalar.dma_start(out=y_v[grp][:, 2:4], in_=ysb_v[:, 2:4])
    nc.compile()
    return nc


def _make_in_maps(x: np.ndarray, scale: int, n_cores: int):
    xf = np.asarray(x, dtype=np.float32).reshape(-1, C)
    W = _build_w(scale)
    return [{"x": _shard_xt(xf, i, n_cores), "w": W} for i in range(n_cores)]


def kernel(x: np.ndarray, scale) -> np.ndarray:
    import sys
    if "/opt/trn_rl_repo" not in sys.path:
        sys.path.insert(0, "/opt/trn_rl_repo")
    from concourse.bass_utils import run_bass_kernel_spmd

    scale = int(np.asarray(scale))
    x = np.asarray(x, dtype=np.float32)
    orig_shape = x.shape

    key = ("nc", N_CORES)
    if key not in _CACHE:
        _CACHE[key] = _build_nc(N_CORES)
    nc = _CACHE[key]

    rows = ROWS_TOTAL // N_CORES
    in_maps = _make_in_maps(x, scale, N_CORES)
    res = run_bass_kernel_spmd(nc, in_maps, list(range(N_CORES)))
    outs = [np.asarray(r["y"][:rows], dtype=np.float32) for r in res.results]
    y = np.concatenate(outs, axis=0).reshape(orig_shape)
    return y
